# revision 1
# baseline (speedup 1.0000x reference)
"""MoE routing kernel for Trainium2 (8 NeuronCores, expert-parallel).

Strategy:
  - Host: compute gate (sigmoid + grouped top-k routing) in numpy, gather
    tokens per expert (sparse dispatch; top-2 of 8 experts per token).
  - Device (SPMD, core e): SwiGLU MLP with expert e's weights over the
    tokens routed to e, plus a 1/8 token-shard of the shared-expert MLP.
    Layout keeps features on SBUF partitions and streams tokens along the
    free axis, so activations feed matmuls without any on-device transpose.
    Matmuls run as float32r (full-rate single-pass, fp32 storage).
  - Host: weighted scatter-add of expert outputs + shared output.
"""

import numpy as np
from contextlib import ExitStack

DIM = 768
INTER = 512
E = 8
G = 4
TOPK = 2
N_CORES = 8
P = 128
NCHUNK = 512  # tokens per PSUM tile (fp32 bank limit)


# ---------------------------------------------------------------- host gate
def _host_gate(x2, gate_weight, gate_bias):
    """Reproduces reference._gate in numpy f32. Returns (w [T,2], idx [T,2])."""
    T = x2.shape[0]
    logits = x2 @ gate_weight.T
    scores = 1.0 / (1.0 + np.exp(-logits, dtype=np.float32))
    s = scores + gate_bias
    sv = s.reshape(T, G, E // G)
    group_scores = sv.sum(-1)  # top-2 of 2 per group == sum
    gidx = np.argsort(-group_scores, axis=1, kind="stable")[:, :2]
    gmask = np.zeros((T, G), bool)
    gmask[np.arange(T)[:, None], gidx] = True
    masked = np.where(gmask[:, :, None], sv, -np.inf).reshape(T, E)
    idx = np.argsort(-masked, axis=1, kind="stable")[:, :TOPK]
    w = np.take_along_axis(scores, idx, axis=1)
    w = w / (w.sum(-1, keepdims=True) + 1e-6)
    return w.astype(np.float32), idx.astype(np.int32)


# ---------------------------------------------------------- device kernel IR
def _build_nc(cap, nsh):
    import concourse.bass as bass
    import concourse.tile as tile
    from concourse import bacc, mybir

    f32 = mybir.dt.float32
    f32r = mybir.dt.float32r
    KD = DIM // P    # 6 k-tiles over model dim
    KI = INTER // P  # 4 k-tiles over inter dim

    nc = bacc.Bacc(
        "TRN2",
        target_bir_lowering=False,
        debug=False,
        enable_asserts=False,
        num_devices=N_CORES,
    )

    xg = nc.dram_tensor("xg", [DIM, cap], f32r, kind="ExternalInput").ap()
    xs = nc.dram_tensor("xs", [DIM, nsh], f32r, kind="ExternalInput").ap()
    w1t = nc.dram_tensor("w1t", [DIM, INTER], f32r, kind="ExternalInput").ap()
    w3t = nc.dram_tensor("w3t", [DIM, INTER], f32r, kind="ExternalInput").ap()
    w2t = nc.dram_tensor("w2t", [INTER, DIM], f32r, kind="ExternalInput").ap()
    sw1t = nc.dram_tensor("sw1t", [DIM, INTER], f32r, kind="ExternalInput").ap()
    sw3t = nc.dram_tensor("sw3t", [DIM, INTER], f32r, kind="ExternalInput").ap()
    sw2t = nc.dram_tensor("sw2t", [INTER, DIM], f32r, kind="ExternalInput").ap()
    oe = nc.dram_tensor("oe", [DIM, cap], f32, kind="ExternalOutput").ap()
    oz = nc.dram_tensor("oz", [DIM, nsh], f32, kind="ExternalOutput").ap()

    with tile.TileContext(nc) as tc, ExitStack() as ctx:
        wpool = ctx.enter_context(tc.tile_pool(name="wpool", bufs=1))
        xpool = ctx.enter_context(tc.tile_pool(name="xpool", bufs=3))
        hpool = ctx.enter_context(tc.tile_pool(name="hpool", bufs=2))
        sgpool = ctx.enter_context(tc.tile_pool(name="sgpool", bufs=2))
        opool = ctx.enter_context(tc.tile_pool(name="opool", bufs=2))
        # p1/p3 get 3 banks each, p2 gets 2: all 8 PSUM banks in play
        ppool = ctx.enter_context(tc.tile_pool(name="ppool", bufs=3, space="PSUM"))
        ppool2 = ctx.enter_context(tc.tile_pool(name="ppool2", bufs=2, space="PSUM"))

        xsr = xs.rearrange("(kt p) n -> p kt n", p=P)

        # Critical-path prologue on the fast sync queue: interleave the first
        # shared x-chunk with the first-needed weight (sw1) per k-tile, so the
        # first matmul starts after ~2 small transfers instead of ~3MB.
        n_first = min(NCHUNK, nsh)
        xt0 = xpool.tile([P, KD, NCHUNK], f32r, tag="xt", name="xt0")
        sw1s = wpool.tile([P, KD, INTER], f32r, tag="sw1s", name="sw1s")
        sw1r = sw1t.rearrange("(kt p) m -> p kt m", p=P)
        for k in range(KD):
            nc.sync.dma_start(out=xt0[:, k, :n_first], in_=xsr[:, k, :n_first])
            nc.sync.dma_start(out=sw1s[:, k, :], in_=sw1r[:, k, :])

        def load_weight(ap_, tag):
            # DRAM [K, M] -> SBUF [P, K//P, M]; lhsT slices are [:, k, m*P:(m+1)*P]
            # per-k-tile DMAs on the gpsimd queue (no competing work there)
            kt = ap_.shape[0] // P
            t = wpool.tile([P, kt, ap_.shape[1]], f32r, tag=tag, name=tag)
            src = ap_.rearrange("(kt p) m -> p kt m", p=P)
            for k in range(kt):
                nc.gpsimd.dma_start(out=t[:, k, :], in_=src[:, k, :])
            return t

        sw3s = load_weight(sw3t, "sw3s")
        sw2s = load_weight(sw2t, "sw2s")
        w1s = load_weight(w1t, "w1s")
        w3s = load_weight(w3t, "w3s")
        w2s = load_weight(w2t, "w2s")

        def swiglu(xT, outT, a1, a3, a2, ntok, xt_pre=None):
            xTr = xT.rearrange("(kt p) n -> p kt n", p=P)
            oTr = outT.rearrange("(kt p) n -> p kt n", p=P)
            nchunks = (ntok + NCHUNK - 1) // NCHUNK
            for c in range(nchunks):
                n0 = c * NCHUNK
                n = min(NCHUNK, ntok - n0)
                if c == 0 and xt_pre is not None:
                    xt = xt_pre
                else:
                    xt = xpool.tile([P, KD, NCHUNK], f32r, tag="xt", name="xt")
                    nc.sync.dma_start(out=xt[:, :, :n], in_=xTr[:, :, n0 : n0 + n])
                h = hpool.tile([P, KI, NCHUNK], f32r, tag="h", name="h")
                for m in range(KI):
                    p1 = ppool.tile([P, NCHUNK], f32, tag="p1", name="p1")
                    for k in range(KD):
                        nc.tensor.matmul(
                            p1[:, :n],
                            a1[:, k, m * P : (m + 1) * P],
                            xt[:, k, :n],
                            start=(k == 0),
                            stop=(k == KD - 1),
                        )
                    # silu(x) = x * sigmoid(x)
                    sg = sgpool.tile([P, NCHUNK], f32, tag="sg", name="sg")
                    nc.scalar.activation(
                        sg[:, :n], p1[:, :n], mybir.ActivationFunctionType.Sigmoid
                    )
                    nc.vector.tensor_mul(h[:, m, :n], sg[:, :n], p1[:, :n])
                    p3 = ppool.tile([P, NCHUNK], f32, tag="p3", name="p3")
                    for k in range(KD):
                        nc.tensor.matmul(
                            p3[:, :n],
                            a3[:, k, m * P : (m + 1) * P],
                            xt[:, k, :n],
                            start=(k == 0),
                            stop=(k == KD - 1),
                        )
                    nc.vector.tensor_mul(h[:, m, :n], h[:, m, :n], p3[:, :n])
                ot = opool.tile([P, KD, NCHUNK], f32, tag="ot", name="ot")
                for m2 in range(KD):
                    p2 = ppool2.tile([P, NCHUNK], f32, tag="p2", name="p2")
                    for k2 in range(KI):
                        nc.tensor.matmul(
                            p2[:, :n],
                            a2[:, k2, m2 * P : (m2 + 1) * P],
                            h[:, k2, :n],
                            start=(k2 == 0),
                            stop=(k2 == KI - 1),
                        )
                    nc.vector.tensor_copy(ot[:, m2, :n], p2[:, :n])
                nc.sync.dma_start(out=oTr[:, :, n0 : n0 + n], in_=ot[:, :, :n])

        # shared phase first: the routed remainder chunk (smallest) drains last
        swiglu(xs, oz, sw1s, sw3s, sw2s, nsh, xt_pre=xt0)
        swiglu(xg, oe, w1s, w3s, w2s, cap)

    nc.compile()
    return nc


# ------------------------------------------------------------------- driver
def kernel(x, gate_weight, gate_bias, w1, w2, w3, sw1, sw2, sw3):
    from concourse.bass_utils import run_bass_kernel_spmd

    B, S, D = x.shape
    x2 = np.ascontiguousarray(x.reshape(-1, D))
    T = x2.shape[0]
    nsh = T // N_CORES

    w, idx = _host_gate(x2, gate_weight, gate_bias)

    rows_per_e = [np.nonzero((idx == e).any(axis=1))[0] for e in range(E)]
    cap = max(len(r) for r in rows_per_e)
    cap = ((cap + P - 1) // P) * P

    nc = _build_nc(cap, nsh)

    x2T = np.ascontiguousarray(x2.T)  # [D, T]
    in_maps = []
    for e in range(E):
        rows = rows_per_e[e]
        xgT = np.zeros((DIM, cap), np.float32)
        xgT[:, : len(rows)] = x2T[:, rows]
        in_maps.append(
            {
                "xg": xgT,
                "xs": np.ascontiguousarray(x2T[:, e * nsh : (e + 1) * nsh]),
                "w1t": np.ascontiguousarray(w1[e].T),
                "w3t": np.ascontiguousarray(w3[e].T),
                "w2t": np.ascontiguousarray(w2[e].T),
                "sw1t": np.ascontiguousarray(sw1.T),
                "sw3t": np.ascontiguousarray(sw3.T),
                "sw2t": np.ascontiguousarray(sw2.T),
            }
        )

    r = run_bass_kernel_spmd(nc, in_maps, list(range(N_CORES)))
    globals()["LAST_RESULTS"] = r
    res = r.results

    y = np.zeros((T, D), np.float32)
    for e in range(E):
        rows = rows_per_e[e]
        cnt = len(rows)
        Oe = res[e]["oe"][:, :cnt].T  # [cnt, D]
        we = np.where(idx[rows, 0] == e, w[rows, 0], w[rows, 1]).astype(np.float32)
        y[rows] += we[:, None] * Oe
    z = np.concatenate([res[c]["oz"].T for c in range(N_CORES)], axis=0)  # [T, D]
    return (y + z).reshape(B, S, D)



# revision 2
# speedup vs baseline: 1.0843x; 1.0843x over previous
"""MoE routing kernel for Trainium2 (8 NeuronCores, expert-parallel).

Strategy:
  - Host: compute gate (sigmoid + grouped top-k routing) in numpy, gather
    tokens per expert (sparse dispatch; top-2 of 8 experts per token).
  - Device (SPMD, core e): SwiGLU MLP with expert e's weights over the
    tokens routed to e, plus a 1/8 token-shard of the shared-expert MLP.
    All matmul operands in bf16 (f32 PSUM accumulation): same PE rate as
    f32r but half the DMA traffic and fully-hidden weight loads.
    Activations/weights are pre-swizzled on host into [128, chunk-major]
    DRAM layout so every DMA is one descriptor with 6KB contiguous per
    partition line.
  - Host: weighted scatter-add of expert outputs + shared output.
"""

import numpy as np
import ml_dtypes
from contextlib import ExitStack

DIM = 768
INTER = 512
E = 8
G = 4
TOPK = 2
N_CORES = 8
P = 128
NCHUNK = 512  # tokens per PSUM tile (fp32 bank limit)
KD = DIM // P  # 6 k-tiles over model dim
KI = INTER // P  # 4 k-tiles over inter dim

BF16 = ml_dtypes.bfloat16


# ---------------------------------------------------------------- host gate
def _host_gate(x2, gate_weight, gate_bias):
    """Reproduces reference._gate in numpy f32. Returns (w [T,2], idx [T,2])."""
    T = x2.shape[0]
    logits = x2 @ gate_weight.T
    scores = 1.0 / (1.0 + np.exp(-logits, dtype=np.float32))
    s = scores + gate_bias
    sv = s.reshape(T, G, E // G)
    group_scores = sv.sum(-1)  # top-2 of 2 per group == sum
    gidx = np.argsort(-group_scores, axis=1, kind="stable")[:, :2]
    gmask = np.zeros((T, G), bool)
    gmask[np.arange(T)[:, None], gidx] = True
    masked = np.where(gmask[:, :, None], sv, -np.inf).reshape(T, E)
    idx = np.argsort(-masked, axis=1, kind="stable")[:, :TOPK]
    w = np.take_along_axis(scores, idx, axis=1)
    w = w / (w.sum(-1, keepdims=True) + 1e-6)
    return w.astype(np.float32), idx.astype(np.int32)


# --------------------------------------------------------- host swizzle utils
def _chunks_of(n):
    """Chunk sizes covering n tokens: full NCHUNKs then one remainder."""
    out = []
    r = n
    while r > 0:
        c = min(NCHUNK, r)
        out.append(c)
        r -= c
    return out


def _swizzle_x(x2T_pad):
    """[DIM, n] (n mult of 128) -> [P, n//NCHUNK groups...] chunk-major bf16.

    Returns array [P, total_free] where chunk c occupies
    [:, c*KD*NCHUNK : (c+1)*KD*NCHUNK], laid out as [KD, chunk] contiguous.
    Last chunk may be short; it is padded to its own size only.
    """
    n = x2T_pad.shape[1]
    sizes = _chunks_of(n)
    pieces = []
    off = 0
    for c in sizes:
        blk = x2T_pad[:, off : off + c]  # [DIM, c]
        blk = blk.reshape(KD, P, c).transpose(1, 0, 2).reshape(P, KD * c)
        pieces.append(blk)
        off += c
    return np.ascontiguousarray(np.concatenate(pieces, axis=1).astype(BF16))


def _swizzle_w(wT):
    """[K, M] weight (torch .T already applied) -> [P, (K//P)*M] bf16."""
    K, M = wT.shape
    kt = K // P
    return np.ascontiguousarray(
        wT.reshape(kt, P, M).transpose(1, 0, 2).reshape(P, kt * M).astype(BF16)
    )


def _unswizzle_out(o, n):
    """[P, sum(KD*c)] bf16 chunk-major -> [n, DIM] f32."""
    sizes = _chunks_of(n)
    cols = []
    off = 0
    o = o.astype(np.float32)
    for c in sizes:
        blk = o[:, off : off + KD * c].reshape(P, KD, c)
        blk = blk.transpose(1, 0, 2).reshape(DIM, c)
        cols.append(blk)
        off += KD * c
    return np.concatenate(cols, axis=1).T  # [n, DIM]


# ---------------------------------------------------------- device kernel IR
def _build_nc(cap, nsh):
    import concourse.bass as bass
    import concourse.tile as tile
    from concourse import bacc, mybir

    f32 = mybir.dt.float32
    bf = mybir.dt.bfloat16

    nc = bacc.Bacc(
        "TRN2",
        target_bir_lowering=False,
        debug=False,
        enable_asserts=False,
        num_devices=N_CORES,
    )

    def xfree(n):
        return sum(KD * c for c in _chunks_of(n))

    xg = nc.dram_tensor("xg", [P, xfree(cap)], bf, kind="ExternalInput").ap()
    xs = nc.dram_tensor("xs", [P, xfree(nsh)], bf, kind="ExternalInput").ap()
    w1t = nc.dram_tensor("w1t", [P, KD * INTER], bf, kind="ExternalInput").ap()
    w3t = nc.dram_tensor("w3t", [P, KD * INTER], bf, kind="ExternalInput").ap()
    w2t = nc.dram_tensor("w2t", [P, KI * DIM], bf, kind="ExternalInput").ap()
    sw1t = nc.dram_tensor("sw1t", [P, KD * INTER], bf, kind="ExternalInput").ap()
    sw3t = nc.dram_tensor("sw3t", [P, KD * INTER], bf, kind="ExternalInput").ap()
    sw2t = nc.dram_tensor("sw2t", [P, KI * DIM], bf, kind="ExternalInput").ap()
    oe = nc.dram_tensor("oe", [P, xfree(cap)], bf, kind="ExternalOutput").ap()
    oz = nc.dram_tensor("oz", [P, xfree(nsh)], bf, kind="ExternalOutput").ap()

    with tile.TileContext(nc) as tc, ExitStack() as ctx:
        wpool = ctx.enter_context(tc.tile_pool(name="wpool", bufs=1))
        xpool = ctx.enter_context(tc.tile_pool(name="xpool", bufs=3))
        hpool = ctx.enter_context(tc.tile_pool(name="hpool", bufs=2))
        sgpool = ctx.enter_context(tc.tile_pool(name="sgpool", bufs=2))
        opool = ctx.enter_context(tc.tile_pool(name="opool", bufs=2))
        # PSUM banks: p1 x3, p3 x2, p2 x3 (8 total)
        p1pool = ctx.enter_context(tc.tile_pool(name="p1pool", bufs=3, space="PSUM"))
        p3pool = ctx.enter_context(tc.tile_pool(name="p3pool", bufs=2, space="PSUM"))
        p2pool = ctx.enter_context(tc.tile_pool(name="p2pool", bufs=3, space="PSUM"))

        # ---- weight tiles (persistent) ----
        def wtile(tag, kt, m):
            return wpool.tile([P, kt, m], bf, tag=tag, name=tag)

        sw1s = wtile("sw1s", KD, INTER)
        sw3s = wtile("sw3s", KD, INTER)
        sw2s = wtile("sw2s", KI, DIM)
        w1s = wtile("w1s", KD, INTER)
        w3s = wtile("w3s", KD, INTER)
        w2s = wtile("w2s", KI, DIM)

        # Critical path: interleave sw1/sw3 k-tiles with the first shared
        # x-chunk on the fast sync queue, k-tile-paired, so matmul k of the
        # first m-group starts as soon as pair k has landed.
        n0 = min(NCHUNK, nsh)
        xt0 = xpool.tile([P, KD, NCHUNK], bf, tag="xt", name="xt0")
        sw1r = sw1t.rearrange("p (kt m) -> p kt m", kt=KD)
        sw3r = sw3t.rearrange("p (kt m) -> p kt m", kt=KD)
        x0r = xs[:, 0 : KD * n0].rearrange("p (kt n) -> p kt n", kt=KD)
        for k in range(KD):
            nc.sync.dma_start(out=sw1s[:, k, :], in_=sw1r[:, k, :])
            nc.sync.dma_start(out=xt0[:, k, :n0], in_=x0r[:, k, :])
            nc.sync.dma_start(out=sw3s[:, k, :], in_=sw3r[:, k, :])

        # Remaining weights on the gpsimd queue, consumption order.
        for t, src in ((sw2s, sw2t), (w1s, w1t), (w3s, w3t), (w2s, w2t)):
            nc.gpsimd.dma_start(
                out=t[:, :, :],
                in_=src.rearrange("p (kt m) -> p kt m", kt=t.shape[1]),
            )

        Silu = mybir.ActivationFunctionType.Silu
        Copy = mybir.ActivationFunctionType.Copy

        def swiglu(xT, outT, a1, a3, a2, ntok, xt_pre=None):
            sizes = _chunks_of(ntok)
            xoff = 0
            for c, n in enumerate(sizes):
                if c == 0 and xt_pre is not None:
                    xt = xt_pre
                else:
                    xt = xpool.tile([P, KD, NCHUNK], bf, tag="xt", name="xt")
                    src = xT[:, xoff : xoff + KD * n].rearrange(
                        "p (kt n) -> p kt n", kt=KD
                    )
                    nc.sync.dma_start(out=xt[:, :, :n], in_=src)
                h = hpool.tile([P, KI, NCHUNK], bf, tag="h", name="h")
                for m in range(KI):
                    p1 = p1pool.tile([P, NCHUNK], f32, tag="p1", name="p1")
                    for k in range(KD):
                        nc.tensor.matmul(
                            p1[:, :n],
                            a1[:, k, m * P : (m + 1) * P],
                            xt[:, k, :n],
                            start=(k == 0),
                            stop=(k == KD - 1),
                        )
                    sg = sgpool.tile([P, NCHUNK], bf, tag="sg", name="sg")
                    nc.scalar.activation(sg[:, :n], p1[:, :n], Silu)
                    p3 = p3pool.tile([P, NCHUNK], f32, tag="p3", name="p3")
                    for k in range(KD):
                        nc.tensor.matmul(
                            p3[:, :n],
                            a3[:, k, m * P : (m + 1) * P],
                            xt[:, k, :n],
                            start=(k == 0),
                            stop=(k == KD - 1),
                        )
                    nc.vector.tensor_mul(h[:, m, :n], sg[:, :n], p3[:, :n])
                ot = opool.tile([P, KD, NCHUNK], bf, tag="ot", name="ot")
                for m2 in range(KD):
                    p2 = p2pool.tile([P, NCHUNK], f32, tag="p2", name="p2")
                    for k2 in range(KI):
                        nc.tensor.matmul(
                            p2[:, :n],
                            a2[:, k2, m2 * P : (m2 + 1) * P],
                            h[:, k2, :n],
                            start=(k2 == 0),
                            stop=(k2 == KI - 1),
                        )
                    nc.scalar.activation(ot[:, m2, :n], p2[:, :n], Copy)
                dst = outT[:, xoff : xoff + KD * n].rearrange(
                    "p (kt n) -> p kt n", kt=KD
                )
                nc.sync.dma_start(out=dst, in_=ot[:, :, :n])
                xoff += KD * n

        # shared phase first: routed weights stream in behind it
        swiglu(xs, oz, sw1s, sw3s, sw2s, nsh, xt_pre=xt0)
        swiglu(xg, oe, w1s, w3s, w2s, cap)

    nc.compile()
    return nc


# ------------------------------------------------------------------- driver
def kernel(x, gate_weight, gate_bias, w1, w2, w3, sw1, sw2, sw3):
    from concourse.bass_utils import run_bass_kernel_spmd

    B, S, D = x.shape
    x2 = np.ascontiguousarray(x.reshape(-1, D))
    T = x2.shape[0]
    nsh = T // N_CORES

    w, idx = _host_gate(x2, gate_weight, gate_bias)

    rows_per_e = [np.nonzero((idx == e).any(axis=1))[0] for e in range(E)]
    cap = max(len(r) for r in rows_per_e)
    cap = ((cap + P - 1) // P) * P

    nc = _build_nc(cap, nsh)

    x2T = np.ascontiguousarray(x2.T)  # [D, T]
    in_maps = []
    for e in range(E):
        rows = rows_per_e[e]
        xgT = np.zeros((DIM, cap), np.float32)
        xgT[:, : len(rows)] = x2T[:, rows]
        in_maps.append(
            {
                "xg": _swizzle_x(xgT),
                "xs": _swizzle_x(x2T[:, e * nsh : (e + 1) * nsh]),
                "w1t": _swizzle_w(w1[e].T),
                "w3t": _swizzle_w(w3[e].T),
                "w2t": _swizzle_w(w2[e].T),
                "sw1t": _swizzle_w(sw1.T),
                "sw3t": _swizzle_w(sw3.T),
                "sw2t": _swizzle_w(sw2.T),
            }
        )

    r = run_bass_kernel_spmd(nc, in_maps, list(range(N_CORES)))
    globals()["LAST_RESULTS"] = r
    res = r.results

    y = np.zeros((T, D), np.float32)
    for e in range(E):
        rows = rows_per_e[e]
        cnt = len(rows)
        Oe = _unswizzle_out(res[e]["oe"], cap)[:cnt]  # [cnt, D]
        we = np.where(idx[rows, 0] == e, w[rows, 0], w[rows, 1]).astype(np.float32)
        y[rows] += we[:, None] * Oe
    z = np.concatenate(
        [_unswizzle_out(res[c]["oz"], nsh) for c in range(N_CORES)], axis=0
    )
    return (y + z).reshape(B, S, D)


# revision 4
# speedup vs baseline: 1.1165x; 1.0297x over previous
"""MoE routing kernel for Trainium2 (8 NeuronCores, expert-parallel).

Strategy:
  - Host: compute gate (sigmoid + grouped top-k routing) in numpy, gather
    tokens per expert (sparse dispatch; top-2 of 8 experts per token).
  - Device (SPMD, core e): SwiGLU MLP with expert e's weights over the
    tokens routed to e, plus a 1/8 token-shard of the shared-expert MLP.
    All matmul operands bf16 (f32 PSUM): same PE rate as f32r, half the
    DMA bytes, fully-hidden weight loads. Host pre-swizzles x/weights/out
    into [128, ...] chunk-major DRAM layouts so every DMA is a single
    contiguous-per-partition descriptor.
  - Warmup burst of dummy matmuls trips the PE HAM clock-gate to 2.4 GHz
    before the first real matmul; first weight m-block + first x half are
    prefetched so real matmuls start ~3us in, already warm.
  - Host: weighted scatter-add of expert outputs + shared output.
"""

import numpy as np
import ml_dtypes
from contextlib import ExitStack

DIM = 768
INTER = 512
E = 8
G = 4
TOPK = 2
N_CORES = 8
P = 128
NCHUNK = 512  # tokens per PSUM tile (fp32 bank limit)
KD = DIM // P  # 6 k-tiles over model dim
KI = INTER // P  # 4 k-tiles over inter dim
WARMUP_MM = 40

BF16 = ml_dtypes.bfloat16


# ---------------------------------------------------------------- host gate
def _host_gate(x2, gate_weight, gate_bias):
    """Reproduces reference._gate in numpy f32. Returns (w [T,2], idx [T,2])."""
    T = x2.shape[0]
    logits = x2 @ gate_weight.T
    scores = 1.0 / (1.0 + np.exp(-logits, dtype=np.float32))
    s = scores + gate_bias
    sv = s.reshape(T, G, E // G)
    group_scores = sv.sum(-1)  # top-2 of 2 per group == sum
    gidx = np.argsort(-group_scores, axis=1, kind="stable")[:, :2]
    gmask = np.zeros((T, G), bool)
    gmask[np.arange(T)[:, None], gidx] = True
    masked = np.where(gmask[:, :, None], sv, -np.inf).reshape(T, E)
    idx = np.argsort(-masked, axis=1, kind="stable")[:, :TOPK]
    w = np.take_along_axis(scores, idx, axis=1)
    w = w / (w.sum(-1, keepdims=True) + 1e-6)
    return w.astype(np.float32), idx.astype(np.int32)


# --------------------------------------------------------- host swizzle utils
def _chunks_of(n):
    """Chunk sizes covering n tokens: full NCHUNKs then one remainder."""
    out = []
    r = n
    while r > 0:
        c = min(NCHUNK, r)
        out.append(c)
        r -= c
    return out


def _swizzle_x(x2T_pad):
    """[DIM, n] -> [P, sum(KD*c)] chunk-major bf16 ([KD, c] per chunk)."""
    n = x2T_pad.shape[1]
    pieces = []
    off = 0
    for c in _chunks_of(n):
        blk = x2T_pad[:, off : off + c]  # [DIM, c]
        blk = blk.reshape(KD, P, c).transpose(1, 0, 2).reshape(P, KD * c)
        pieces.append(blk)
        off += c
    return np.ascontiguousarray(np.concatenate(pieces, axis=1).astype(BF16))


def _swizzle_w13(wT):
    """[DIM, INTER] (w.T) -> [P, KI, KD, P] m-major bf16, flattened."""
    a = wT.reshape(KD, P, KI, P).transpose(1, 2, 0, 3)  # [P, KI, KD, P]
    return np.ascontiguousarray(a.reshape(P, KI * KD * P).astype(BF16))


def _swizzle_w2(w2T):
    """[INTER, DIM] (w2.T) -> [P, KD, KI, P] m-major bf16, flattened."""
    a = w2T.reshape(KI, P, KD, P).transpose(1, 2, 0, 3)  # [P, KD, KI, P]
    return np.ascontiguousarray(a.reshape(P, KD * KI * P).astype(BF16))


def _unswizzle_out(o, n):
    """[P, sum(KD*c)] bf16 chunk-major -> [n, DIM] f32."""
    cols = []
    off = 0
    o = o.astype(np.float32)
    for c in _chunks_of(n):
        blk = o[:, off : off + KD * c].reshape(P, KD, c)
        blk = blk.transpose(1, 0, 2).reshape(DIM, c)
        cols.append(blk)
        off += KD * c
    return np.concatenate(cols, axis=1).T  # [n, DIM]


# ---------------------------------------------------------- device kernel IR
def _build_nc(cap, nsh):
    import concourse.bass as bass
    import concourse.tile as tile
    from concourse import bacc, mybir

    f32 = mybir.dt.float32
    bf = mybir.dt.bfloat16

    nc = bacc.Bacc(
        "TRN2",
        target_bir_lowering=False,
        debug=False,
        enable_asserts=False,
        num_devices=N_CORES,
    )

    def xfree(n):
        return sum(KD * c for c in _chunks_of(n))

    xg = nc.dram_tensor("xg", [P, xfree(cap)], bf, kind="ExternalInput").ap()
    xs = nc.dram_tensor("xs", [P, xfree(nsh)], bf, kind="ExternalInput").ap()
    w1t = nc.dram_tensor("w1t", [P, KI * KD * P], bf, kind="ExternalInput").ap()
    w3t = nc.dram_tensor("w3t", [P, KI * KD * P], bf, kind="ExternalInput").ap()
    w2t = nc.dram_tensor("w2t", [P, KD * KI * P], bf, kind="ExternalInput").ap()
    sw1t = nc.dram_tensor("sw1t", [P, KI * KD * P], bf, kind="ExternalInput").ap()
    sw3t = nc.dram_tensor("sw3t", [P, KI * KD * P], bf, kind="ExternalInput").ap()
    sw2t = nc.dram_tensor("sw2t", [P, KD * KI * P], bf, kind="ExternalInput").ap()
    oe = nc.dram_tensor("oe", [P, xfree(cap)], bf, kind="ExternalOutput").ap()
    oz = nc.dram_tensor("oz", [P, xfree(nsh)], bf, kind="ExternalOutput").ap()

    with tile.TileContext(nc) as tc, ExitStack() as ctx:
        wpool = ctx.enter_context(tc.tile_pool(name="wpool", bufs=1))
        xpool = ctx.enter_context(tc.tile_pool(name="xpool", bufs=3))
        hpool = ctx.enter_context(tc.tile_pool(name="hpool", bufs=3))
        sgpool = ctx.enter_context(tc.tile_pool(name="sgpool", bufs=3))
        opool = ctx.enter_context(tc.tile_pool(name="opool", bufs=3))
        # PSUM banks: p1 x3, p3 x2, p2 x3 (8 total)
        p1pool = ctx.enter_context(tc.tile_pool(name="p1pool", bufs=3, space="PSUM"))
        p3pool = ctx.enter_context(tc.tile_pool(name="p3pool", bufs=2, space="PSUM"))
        p2pool = ctx.enter_context(tc.tile_pool(name="p2pool", bufs=3, space="PSUM"))

        # ---- HAM warmup: dummy matmuls to trip the PE clock to 2.4 GHz ----
        wz = wpool.tile([P, 64], bf, tag="wz", name="wz")
        nc.gpsimd.memset(wz, 0.0)
        pwarm = p2pool.tile([P, NCHUNK], f32, tag="p2", name="pwarm")
        for i in range(WARMUP_MM):
            nc.tensor.matmul(
                pwarm[:64, :64],
                wz[:, :64],
                wz[:, :64],
                start=(i == 0),
                stop=(i == WARMUP_MM - 1),
            )

        # ---- weight tiles (persistent, m-major) ----
        sw1s = wpool.tile([P, KI, KD, P], bf, tag="sw1s", name="sw1s")
        sw3s = wpool.tile([P, KI, KD, P], bf, tag="sw3s", name="sw3s")
        sw2s = wpool.tile([P, KD, KI, P], bf, tag="sw2s", name="sw2s")
        w1s = wpool.tile([P, KI, KD, P], bf, tag="w1s", name="w1s")
        w3s = wpool.tile([P, KI, KD, P], bf, tag="w3s", name="w3s")
        w2s = wpool.tile([P, KD, KI, P], bf, tag="w2s", name="w2s")

        # Shared-expert weights m-block-interleaved on gpsimd queue: the
        # first matmul group only needs sw1 m-block 0 + half of x chunk 0.
        sw1r = sw1t.rearrange("p (mb k mi) -> p mb k mi", mb=KI, k=KD)
        sw3r = sw3t.rearrange("p (mb k mi) -> p mb k mi", mb=KI, k=KD)
        for mb in range(KI):
            nc.gpsimd.dma_start(out=sw1s[:, mb], in_=sw1r[:, mb])
            nc.gpsimd.dma_start(out=sw3s[:, mb], in_=sw3r[:, mb])
        nc.gpsimd.dma_start(
            out=sw2s[:, :, :, :],
            in_=sw2t.rearrange("p (mb k mi) -> p mb k mi", mb=KD, k=KI),
        )
        # Routed weights: whole-tensor loads, needed only after shared phase.
        for t, src, mbn, kn in (
            (w1s, w1t, KI, KD),
            (w3s, w3t, KI, KD),
            (w2s, w2t, KD, KI),
        ):
            nc.gpsimd.dma_start(
                out=t[:, :, :, :],
                in_=src.rearrange("p (mb k mi) -> p mb k mi", mb=mbn, k=kn),
            )

        Silu = mybir.ActivationFunctionType.Silu
        Copy = mybir.ActivationFunctionType.Copy

        # ---- unified chunk schedule over both phases ----
        sched = []  # (xT, outT, a1, a3, a2, xoff, n)
        xoff = 0
        for n in _chunks_of(nsh):
            sched.append((xs, oz, sw1s, sw3s, sw2s, xoff, n))
            xoff += KD * n
        xoff = 0
        for n in _chunks_of(cap):
            sched.append((xg, oe, w1s, w3s, w2s, xoff, n))
            xoff += KD * n

        nchunks = len(sched)
        xtiles = [None] * nchunks

        def issue_x(c, split=False):
            xT, _, _, _, _, xoff, n = sched[c]
            xt = xpool.tile([P, KD, NCHUNK], bf, tag="xt", name="xt")
            src = xT[:, xoff : xoff + KD * n].rearrange("p (kt n) -> p kt n", kt=KD)
            if split:
                h = KD // 2
                nc.sync.dma_start(out=xt[:, :h, :n], in_=src[:, :h, :])
                nc.sync.dma_start(out=xt[:, h:, :n], in_=src[:, h:, :])
            else:
                nc.sync.dma_start(out=xt[:, :, :n], in_=src)
            xtiles[c] = xt

        issue_x(0, split=True)
        issue_x(1)
        next_x = 2

        for c in range(nchunks):
            _, outT, a1, a3, a2, xoff, n = sched[c]
            if next_x < nchunks and next_x <= c + 2:
                issue_x(next_x)
                next_x += 1
            xt = xtiles[c]
            xtiles[c] = None
            h = hpool.tile([P, KI, NCHUNK], bf, tag="h", name="h")
            for m in range(KI):
                p1 = p1pool.tile([P, NCHUNK], f32, tag="p1", name="p1")
                for k in range(KD):
                    nc.tensor.matmul(
                        p1[:, :n],
                        a1[:, m, k, :],
                        xt[:, k, :n],
                        start=(k == 0),
                        stop=(k == KD - 1),
                    )
                sg = sgpool.tile([P, NCHUNK], bf, tag="sg", name="sg")
                nc.scalar.activation(sg[:, :n], p1[:, :n], Silu)
                p3 = p3pool.tile([P, NCHUNK], f32, tag="p3", name="p3")
                for k in range(KD):
                    nc.tensor.matmul(
                        p3[:, :n],
                        a3[:, m, k, :],
                        xt[:, k, :n],
                        start=(k == 0),
                        stop=(k == KD - 1),
                    )
                nc.vector.tensor_mul(h[:, m, :n], sg[:, :n], p3[:, :n])
            ot = opool.tile([P, KD, NCHUNK], bf, tag="ot", name="ot")
            for m2 in range(KD):
                p2 = p2pool.tile([P, NCHUNK], f32, tag="p2", name="p2")
                for k2 in range(KI):
                    nc.tensor.matmul(
                        p2[:, :n],
                        a2[:, m2, k2, :],
                        h[:, k2, :n],
                        start=(k2 == 0),
                        stop=(k2 == KI - 1),
                    )
                nc.scalar.activation(ot[:, m2, :n], p2[:, :n], Copy)
            dst = outT[:, xoff : xoff + KD * n].rearrange("p (kt n) -> p kt n", kt=KD)
            nc.sync.dma_start(out=dst, in_=ot[:, :, :n])

    nc.compile()
    return nc


# ------------------------------------------------------------------- driver
def kernel(x, gate_weight, gate_bias, w1, w2, w3, sw1, sw2, sw3):
    from concourse.bass_utils import run_bass_kernel_spmd

    B, S, D = x.shape
    x2 = np.ascontiguousarray(x.reshape(-1, D))
    T = x2.shape[0]
    nsh = T // N_CORES

    w, idx = _host_gate(x2, gate_weight, gate_bias)

    rows_per_e = [np.nonzero((idx == e).any(axis=1))[0] for e in range(E)]
    cap = max(len(r) for r in rows_per_e)
    cap = ((cap + P - 1) // P) * P

    nc = _build_nc(cap, nsh)

    x2T = np.ascontiguousarray(x2.T)  # [D, T]
    in_maps = []
    for e in range(E):
        rows = rows_per_e[e]
        xgT = np.zeros((DIM, cap), np.float32)
        xgT[:, : len(rows)] = x2T[:, rows]
        in_maps.append(
            {
                "xg": _swizzle_x(xgT),
                "xs": _swizzle_x(x2T[:, e * nsh : (e + 1) * nsh]),
                "w1t": _swizzle_w13(w1[e].T),
                "w3t": _swizzle_w13(w3[e].T),
                "w2t": _swizzle_w2(w2[e].T),
                "sw1t": _swizzle_w13(sw1.T),
                "sw3t": _swizzle_w13(sw3.T),
                "sw2t": _swizzle_w2(sw2.T),
            }
        )

    r = run_bass_kernel_spmd(nc, in_maps, list(range(N_CORES)))
    globals()["LAST_RESULTS"] = r
    res = r.results

    y = np.zeros((T, D), np.float32)
    for e in range(E):
        rows = rows_per_e[e]
        cnt = len(rows)
        Oe = _unswizzle_out(res[e]["oe"], cap)[:cnt]  # [cnt, D]
        we = np.where(idx[rows, 0] == e, w[rows, 0], w[rows, 1]).astype(np.float32)
        y[rows] += we[:, None] * Oe
    z = np.concatenate(
        [_unswizzle_out(res[c]["oz"], nsh) for c in range(N_CORES)], axis=0
    )
    return (y + z).reshape(B, S, D)


# revision 5
# speedup vs baseline: 1.1285x; 1.0108x over previous
"""MoE routing kernel for Trainium2 (8 NeuronCores, expert-parallel).

Strategy:
  - Host: compute gate (sigmoid + grouped top-k routing) in numpy, gather
    tokens per expert (sparse dispatch; top-2 of 8 experts per token).
  - Device (SPMD, core e): SwiGLU MLP with expert e's weights over the
    tokens routed to e, plus a 1/8 token-shard of the shared-expert MLP.
    All matmul operands bf16 (f32 PSUM): same PE rate as f32r, half the
    DMA bytes, fully-hidden weight loads. Host pre-swizzles x/weights/out
    into [128, ...] chunk-major DRAM layouts so every DMA is a single
    contiguous-per-partition descriptor.
  - Software pipeline: the w2 phase is emitted k2-major, one h-column
    behind production, so the tensor engine never waits on the
    silu*h3 vector op (PSUM: p1 x1, p3 x1, p2 x6 banks).
  - DMA need-ordering: critical shared weights first, x prefetch behind
    them on the same queue (all DMA shares 16 engines ~360 GB/s).
  - Warmup burst of dummy matmuls trips the PE HAM clock-gate to 2.4 GHz
    during the DMA head, so real matmuls start at full clock.
  - Host: weighted scatter-add of expert outputs + shared output.
"""

import numpy as np
import ml_dtypes
from contextlib import ExitStack

DIM = 768
INTER = 512
E = 8
G = 4
TOPK = 2
N_CORES = 8
P = 128
NCHUNK = 512  # tokens per PSUM tile (fp32 bank limit)
KD = DIM // P  # 6 k-tiles over model dim
KI = INTER // P  # 4 k-tiles over inter dim
WARMUP_MM = 56

BF16 = ml_dtypes.bfloat16


# ---------------------------------------------------------------- host gate
def _host_gate(x2, gate_weight, gate_bias):
    """Reproduces reference._gate in numpy f32. Returns (w [T,2], idx [T,2])."""
    T = x2.shape[0]
    logits = x2 @ gate_weight.T
    scores = 1.0 / (1.0 + np.exp(-logits, dtype=np.float32))
    s = scores + gate_bias
    sv = s.reshape(T, G, E // G)
    group_scores = sv.sum(-1)  # top-2 of 2 per group == sum
    gidx = np.argsort(-group_scores, axis=1, kind="stable")[:, :2]
    gmask = np.zeros((T, G), bool)
    gmask[np.arange(T)[:, None], gidx] = True
    masked = np.where(gmask[:, :, None], sv, -np.inf).reshape(T, E)
    idx = np.argsort(-masked, axis=1, kind="stable")[:, :TOPK]
    w = np.take_along_axis(scores, idx, axis=1)
    w = w / (w.sum(-1, keepdims=True) + 1e-6)
    return w.astype(np.float32), idx.astype(np.int32)


# --------------------------------------------------------- host swizzle utils
def _chunks_of(n):
    """Chunk sizes covering n tokens: full NCHUNKs then one remainder."""
    out = []
    r = n
    while r > 0:
        c = min(NCHUNK, r)
        out.append(c)
        r -= c
    return out


def _swizzle_x(x2T_pad):
    """[DIM, n] -> [P, sum(KD*c)] chunk-major bf16 ([KD, c] per chunk)."""
    n = x2T_pad.shape[1]
    pieces = []
    off = 0
    for c in _chunks_of(n):
        blk = x2T_pad[:, off : off + c]  # [DIM, c]
        blk = blk.reshape(KD, P, c).transpose(1, 0, 2).reshape(P, KD * c)
        pieces.append(blk)
        off += c
    return np.ascontiguousarray(np.concatenate(pieces, axis=1).astype(BF16))


def _swizzle_w13(wT):
    """[DIM, INTER] (w.T) -> [P, KI, KD, P] m-major bf16, flattened."""
    a = wT.reshape(KD, P, KI, P).transpose(1, 2, 0, 3)  # [P, KI, KD, P]
    return np.ascontiguousarray(a.reshape(P, KI * KD * P).astype(BF16))


def _swizzle_w2(w2T):
    """[INTER, DIM] (w2.T) -> [P, KD, KI, P] m-major bf16, flattened."""
    a = w2T.reshape(KI, P, KD, P).transpose(1, 2, 0, 3)  # [P, KD, KI, P]
    return np.ascontiguousarray(a.reshape(P, KD * KI * P).astype(BF16))


def _unswizzle_out(o, n):
    """[P, sum(KD*c)] bf16 chunk-major -> [n, DIM] f32."""
    cols = []
    off = 0
    o = o.astype(np.float32)
    for c in _chunks_of(n):
        blk = o[:, off : off + KD * c].reshape(P, KD, c)
        blk = blk.transpose(1, 0, 2).reshape(DIM, c)
        cols.append(blk)
        off += KD * c
    return np.concatenate(cols, axis=1).T  # [n, DIM]


# ---------------------------------------------------------- device kernel IR
def _build_nc(cap, nsh):
    import concourse.bass as bass
    import concourse.tile as tile
    from concourse import bacc, mybir

    f32 = mybir.dt.float32
    bf = mybir.dt.bfloat16

    nc = bacc.Bacc(
        "TRN2",
        target_bir_lowering=False,
        debug=False,
        enable_asserts=False,
        num_devices=N_CORES,
    )

    def xfree(n):
        return sum(KD * c for c in _chunks_of(n))

    xg = nc.dram_tensor("xg", [P, xfree(cap)], bf, kind="ExternalInput").ap()
    xs = nc.dram_tensor("xs", [P, xfree(nsh)], bf, kind="ExternalInput").ap()
    w1t = nc.dram_tensor("w1t", [P, KI * KD * P], bf, kind="ExternalInput").ap()
    w3t = nc.dram_tensor("w3t", [P, KI * KD * P], bf, kind="ExternalInput").ap()
    w2t = nc.dram_tensor("w2t", [P, KD * KI * P], bf, kind="ExternalInput").ap()
    sw1t = nc.dram_tensor("sw1t", [P, KI * KD * P], bf, kind="ExternalInput").ap()
    sw3t = nc.dram_tensor("sw3t", [P, KI * KD * P], bf, kind="ExternalInput").ap()
    sw2t = nc.dram_tensor("sw2t", [P, KD * KI * P], bf, kind="ExternalInput").ap()
    oe = nc.dram_tensor("oe", [P, xfree(cap)], bf, kind="ExternalOutput").ap()
    oz = nc.dram_tensor("oz", [P, xfree(nsh)], bf, kind="ExternalOutput").ap()

    with tile.TileContext(nc) as tc, ExitStack() as ctx:
        wpool = ctx.enter_context(tc.tile_pool(name="wpool", bufs=1))
        xpool = ctx.enter_context(tc.tile_pool(name="xpool", bufs=4))
        hpool = ctx.enter_context(tc.tile_pool(name="hpool", bufs=3))
        sgpool = ctx.enter_context(tc.tile_pool(name="sgpool", bufs=3))
        opool = ctx.enter_context(tc.tile_pool(name="opool", bufs=3))
        # PSUM banks: p1 x1, p3 x1, p2 x6 (8 total)
        p1pool = ctx.enter_context(tc.tile_pool(name="p1pool", bufs=1, space="PSUM"))
        p3pool = ctx.enter_context(tc.tile_pool(name="p3pool", bufs=1, space="PSUM"))
        p2pool = ctx.enter_context(tc.tile_pool(name="p2pool", bufs=6, space="PSUM"))

        # ---- HAM warmup: dummy matmuls to trip the PE clock to 2.4 GHz ----
        wz = wpool.tile([P, 64], bf, tag="wz", name="wz")
        nc.vector.memset(wz, 0.0)
        pwarm = p2pool.tile([P, NCHUNK], f32, tag="p2", name="pwarm")
        for i in range(WARMUP_MM):
            nc.tensor.matmul(
                pwarm[:64, :64],
                wz[:, :64],
                wz[:, :64],
                start=(i == 0),
                stop=(i == WARMUP_MM - 1),
            )

        # ---- weight tiles (persistent, m-major) ----
        sw1s = wpool.tile([P, KI, KD, P], bf, tag="sw1s", name="sw1s")
        sw3s = wpool.tile([P, KI, KD, P], bf, tag="sw3s", name="sw3s")
        sw2s = wpool.tile([P, KD, KI, P], bf, tag="sw2s", name="sw2s")
        w1s = wpool.tile([P, KI, KD, P], bf, tag="w1s", name="w1s")
        w3s = wpool.tile([P, KI, KD, P], bf, tag="w3s", name="w3s")
        w2s = wpool.tile([P, KD, KI, P], bf, tag="w2s", name="w2s")

        # ---- unified chunk schedule over both phases ----
        sched = []  # (xT, outT, a1, a3, a2, xoff, n)
        xoff = 0
        for n in _chunks_of(nsh):
            sched.append((xs, oz, sw1s, sw3s, sw2s, xoff, n))
            xoff += KD * n
        xoff = 0
        for n in _chunks_of(cap):
            sched.append((xg, oe, w1s, w3s, w2s, xoff, n))
            xoff += KD * n
        nchunks = len(sched)

        # ---- DMA issue, need-ordered ----
        # x chunk 0 on the sync queue (2 halves); everything else on the
        # gpsimd queue in consumption order: shared-weight m-blocks first,
        # then sw2, x1, x2, routed weights, then the x tail.  All queues
        # feed the same 16 DMA engines; per-queue FIFO order is what
        # controls which bytes land first.
        xtiles = [None] * nchunks

        def issue_x(c, queue, split=False):
            xT, _, _, _, _, xoff, n = sched[c]
            xt = xpool.tile([P, KD, NCHUNK], bf, tag="xt", name="xt")
            src = xT[:, xoff : xoff + KD * n].rearrange("p (kt n) -> p kt n", kt=KD)
            if split:
                hh = KD // 2
                queue.dma_start(out=xt[:, :hh, :n], in_=src[:, :hh, :])
                queue.dma_start(out=xt[:, hh:, :n], in_=src[:, hh:, :])
            else:
                queue.dma_start(out=xt[:, :, :n], in_=src)
            xtiles[c] = xt

        issue_x(0, nc.sync, split=True)

        sw1r = sw1t.rearrange("p (mb k mi) -> p mb k mi", mb=KI, k=KD)
        sw3r = sw3t.rearrange("p (mb k mi) -> p mb k mi", mb=KI, k=KD)
        for mb in range(KI):
            nc.gpsimd.dma_start(out=sw1s[:, mb], in_=sw1r[:, mb])
            nc.gpsimd.dma_start(out=sw3s[:, mb], in_=sw3r[:, mb])
        nc.gpsimd.dma_start(
            out=sw2s[:, :, :, :],
            in_=sw2t.rearrange("p (mb k mi) -> p mb k mi", mb=KD, k=KI),
        )
        issue_x(1, nc.gpsimd)
        issue_x(2, nc.gpsimd)
        for t, src, mbn, kn in (
            (w1s, w1t, KI, KD),
            (w3s, w3t, KI, KD),
            (w2s, w2t, KD, KI),
        ):
            nc.gpsimd.dma_start(
                out=t[:, :, :, :],
                in_=src.rearrange("p (mb k mi) -> p mb k mi", mb=mbn, k=kn),
            )
        for c in range(3, nchunks):
            issue_x(c, nc.gpsimd)

        Silu = mybir.ActivationFunctionType.Silu
        Copy = mybir.ActivationFunctionType.Copy

        htiles = [None] * nchunks
        p2ts = [None] * nchunks

        def emit_batch(c, k2):
            """k2-th accumulation slice into all KD p2 banks of chunk c."""
            _, outT, _, _, a2, xoff, n = sched[c]
            if p2ts[c] is None:
                p2ts[c] = [
                    p2pool.tile([P, NCHUNK], f32, tag="p2", name="p2")
                    for _ in range(KD)
                ]
            h = htiles[c]
            for m2 in range(KD):
                nc.tensor.matmul(
                    p2ts[c][m2][:, :n],
                    a2[:, m2, k2, :],
                    h[:, k2, :n],
                    start=(k2 == 0),
                    stop=(k2 == KI - 1),
                )
            if k2 == KI - 1:
                ot = opool.tile([P, KD, NCHUNK], bf, tag="ot", name="ot")
                for m2 in range(KD):
                    if m2 % 2 == 0:
                        nc.scalar.activation(
                            ot[:, m2, :n], p2ts[c][m2][:, :n], Copy
                        )
                    else:
                        nc.vector.tensor_copy(ot[:, m2, :n], p2ts[c][m2][:, :n])
                dst = outT[:, xoff : xoff + KD * n].rearrange(
                    "p (kt n) -> p kt n", kt=KD
                )
                nc.sync.dma_start(out=dst, in_=ot[:, :, :n])
                htiles[c] = None
                p2ts[c] = None

        prev = None  # (c, m) one h-step behind
        for c in range(nchunks):
            _, _, a1, a3, _, _, n = sched[c]
            xt = xtiles[c]
            htiles[c] = hpool.tile([P, KI, NCHUNK], bf, tag="h", name="h")
            for m in range(KI):
                p1 = p1pool.tile([P, NCHUNK], f32, tag="p1", name="p1")
                for k in range(KD):
                    nc.tensor.matmul(
                        p1[:, :n],
                        a1[:, m, k, :],
                        xt[:, k, :n],
                        start=(k == 0),
                        stop=(k == KD - 1),
                    )
                sg = sgpool.tile([P, NCHUNK], bf, tag="sg", name="sg")
                nc.scalar.activation(sg[:, :n], p1[:, :n], Silu)
                p3 = p3pool.tile([P, NCHUNK], f32, tag="p3", name="p3")
                for k in range(KD):
                    nc.tensor.matmul(
                        p3[:, :n],
                        a3[:, m, k, :],
                        xt[:, k, :n],
                        start=(k == 0),
                        stop=(k == KD - 1),
                    )
                nc.vector.tensor_mul(
                    htiles[c][:, m, :n], sg[:, :n], p3[:, :n]
                )
                if prev is not None:
                    emit_batch(*prev)
                prev = (c, m)
            xtiles[c] = None
        emit_batch(*prev)

    nc.compile()
    return nc


# ------------------------------------------------------------------- driver
def kernel(x, gate_weight, gate_bias, w1, w2, w3, sw1, sw2, sw3):
    from concourse.bass_utils import run_bass_kernel_spmd

    B, S, D = x.shape
    x2 = np.ascontiguousarray(x.reshape(-1, D))
    T = x2.shape[0]
    nsh = T // N_CORES

    w, idx = _host_gate(x2, gate_weight, gate_bias)

    rows_per_e = [np.nonzero((idx == e).any(axis=1))[0] for e in range(E)]
    cap = max(len(r) for r in rows_per_e)
    cap = ((cap + P - 1) // P) * P

    nc = _build_nc(cap, nsh)

    x2T = np.ascontiguousarray(x2.T)  # [D, T]
    in_maps = []
    for e in range(E):
        rows = rows_per_e[e]
        xgT = np.zeros((DIM, cap), np.float32)
        xgT[:, : len(rows)] = x2T[:, rows]
        in_maps.append(
            {
                "xg": _swizzle_x(xgT),
                "xs": _swizzle_x(x2T[:, e * nsh : (e + 1) * nsh]),
                "w1t": _swizzle_w13(w1[e].T),
                "w3t": _swizzle_w13(w3[e].T),
                "w2t": _swizzle_w2(w2[e].T),
                "sw1t": _swizzle_w13(sw1.T),
                "sw3t": _swizzle_w13(sw3.T),
                "sw2t": _swizzle_w2(sw2.T),
            }
        )

    r = run_bass_kernel_spmd(nc, in_maps, list(range(N_CORES)))
    globals()["LAST_RESULTS"] = r
    res = r.results

    y = np.zeros((T, D), np.float32)
    for e in range(E):
        rows = rows_per_e[e]
        cnt = len(rows)
        Oe = _unswizzle_out(res[e]["oe"], cap)[:cnt]  # [cnt, D]
        we = np.where(idx[rows, 0] == e, w[rows, 0], w[rows, 1]).astype(np.float32)
        y[rows] += we[:, None] * Oe
    z = np.concatenate(
        [_unswizzle_out(res[c]["oz"], nsh) for c in range(N_CORES)], axis=0
    )
    return (y + z).reshape(B, S, D)
